# revision 18
# baseline (speedup 1.0000x reference)
"""GaussianMLP sampling kernel for 8 trn2 NeuronCores (pure data parallel).

reference:
    h      = relu(x @ W_emb + b_emb)        x:[B,128] W_emb:[128,256]
    mean   = h @ W_mean + b_mean            W_mean:[256,128]
    logvar = h @ W_logvar + b_logvar        W_logvar:[256,128]
    z      = mean + exp(0.5*logvar) * eps
    returns (z, mean, logvar)

Sharding: x/eps split along batch across 8 cores; weights replicated.

Per-core dataflow (ROWS_PER_TILE=512 rows/iteration, row = t*512 + p*4 + r so
every DMA moves 2KiB-contiguous chunks per partition):
  - DMA x tile [128p, 4, 128] (partition p holds 4 consecutive rows)
  - PE transpose 4x [128,128] -> xT [d_in=128p, 512 rows] in PSUM, DVE copy
    to SBUF as bf16
  - hT = W_emb.T @ x.T via 2 matmuls (lhsT=W_emb chunk, rhs=xT) -> PSUM
  - ACT relu(hT + b_emb) PSUM->SBUF bf16 (per-partition bias)
  - L2: one [128, 2*128] matmul pair per 128-row chunk against
    W_cat = [W_mean | W_logvar] -> cat PSUM [128, (r, head, d)]
  - epilogue: DVE mean/logvar = cat + bias (bias tiles precomputed on host,
    replicated across partitions); ACT std = exp(0.5*logvar); Pool se=std*eps,
    z=mean+se; DMA 3 outputs (2KiB contiguous per partition)
"""

import sys

sys.path.insert(0, "/opt/trn_rl_repo")

import ml_dtypes
import numpy as np

from contextlib import ExitStack

from concourse import bacc, bass, masks, mybir, tile
from concourse.bass_utils import run_bass_kernel_spmd

N_CORES = 8
B = 524288
D_IN = 128
D_H = 256
D_OUT = 128
ROWS_PER_CORE = B // N_CORES  # 65536
ROWS_PER_TILE = 512
N_TILES = ROWS_PER_CORE // ROWS_PER_TILE  # 128
R = ROWS_PER_TILE // 128  # 4 rows per partition per tile

F32 = mybir.dt.float32
BF16 = mybir.dt.bfloat16
L1_DT = BF16
L2_DT = BF16


def build_bass(rows_per_core=ROWS_PER_CORE):
    nc = bacc.Bacc("TRN2", target_bir_lowering=False, debug=False)
    n_tiles = rows_per_core // ROWS_PER_TILE

    x_ext = nc.declare_dram_parameter("x", [rows_per_core, D_IN], BF16, isOutput=False)
    eps_ext = nc.declare_dram_parameter(
        "eps", [rows_per_core, D_OUT], BF16, isOutput=False
    )
    We_ext = nc.declare_dram_parameter("W_emb", [D_IN, D_H], F32, isOutput=False)
    be_ext = nc.declare_dram_parameter("b_emb", [D_H], F32, isOutput=False)
    Wm_ext = nc.declare_dram_parameter("W_mean", [D_H, D_OUT], F32, isOutput=False)
    Wl_ext = nc.declare_dram_parameter("W_logvar", [D_H, D_OUT], F32, isOutput=False)
    # bias rows replicated across 128 partitions and R row-slots (host-prepped)
    bm_rep_ext = nc.declare_dram_parameter(
        "bm_rep", [128, R * D_OUT], F32, isOutput=False
    )
    bl_rep_ext = nc.declare_dram_parameter(
        "bl_rep", [128, R * D_OUT], F32, isOutput=False
    )
    z_ext = nc.declare_dram_parameter("z", [rows_per_core, D_OUT], BF16, isOutput=True)
    mean_ext = nc.declare_dram_parameter(
        "mean", [rows_per_core, D_OUT], BF16, isOutput=True
    )
    lv_ext = nc.declare_dram_parameter(
        "logvar", [rows_per_core, D_OUT], BF16, isOutput=True
    )

    # tiled DRAM views: row = t*ROWS_PER_TILE + p*R + r  (2KiB/partition DMAs)
    xv = x_ext.rearrange("(t p r) d -> t p r d", p=128, r=R)
    ev = eps_ext.rearrange("(t p r) d -> t p (r d)", p=128, r=R)
    zv = z_ext.rearrange("(t p r) d -> t p (r d)", p=128, r=R)
    mv = mean_ext.rearrange("(t p r) d -> t p (r d)", p=128, r=R)
    lvv = lv_ext.rearrange("(t p r) d -> t p (r d)", p=128, r=R)

    with tile.TileContext(nc) as tc, ExitStack() as ctx:
        const = ctx.enter_context(tc.tile_pool(name="const", bufs=1))
        xin = ctx.enter_context(tc.tile_pool(name="xin", bufs=4))
        epool = ctx.enter_context(tc.tile_pool(name="eps", bufs=4))
        xTp = ctx.enter_context(tc.tile_pool(name="xT", bufs=3))
        hTp = ctx.enter_context(tc.tile_pool(name="hTs", bufs=3))
        outs = ctx.enter_context(tc.tile_pool(name="outs", bufs=4))
        psB = ctx.enter_context(tc.tile_pool(name="psB", bufs=2, space="PSUM"))
        psC = ctx.enter_context(tc.tile_pool(name="psC", bufs=2, space="PSUM"))

        # --- constants / weights (loaded once) ---
        We_sb = const.tile([128, D_H], L1_DT)
        nc.gpsimd.dma_start(We_sb[:], We_ext[:])

        # W_cat[:, c, 0:128] = W_mean chunk c, [:, c, 128:256] = W_logvar chunk c
        Wcat_sb = const.tile([128, 2, 2 * D_OUT], L2_DT)
        nc.gpsimd.dma_start(
            Wcat_sb[:, :, 0:D_OUT], Wm_ext.rearrange("(c p) d -> p c d", p=128)
        )
        nc.gpsimd.dma_start(
            Wcat_sb[:, :, D_OUT : 2 * D_OUT],
            Wl_ext.rearrange("(c p) d -> p c d", p=128),
        )

        be_sb = const.tile([128, 2], F32)
        nc.sync.dma_start(be_sb[:], be_ext.rearrange("(c p) -> p c", p=128))

        bm_sb = const.tile([128, R * D_OUT], F32)
        nc.sync.dma_start(bm_sb[:], bm_rep_ext[:])
        bl_sb = const.tile([128, R * D_OUT], F32)
        nc.sync.dma_start(bl_sb[:], bl_rep_ext[:])

        def epilogue(t, cat_ps, eps_sb):
            mean_sb = outs.tile([128, R * D_OUT], BF16, tag="means")
            nc.vector.tensor_add(
                mean_sb[:].rearrange("p (r d) -> p r d", r=R),
                cat_ps[:, :, 0, :],
                bm_sb[:].rearrange("p (r d) -> p r d", r=R),
            )
            lv_sb = outs.tile([128, R * D_OUT], BF16, tag="lvs")
            nc.vector.tensor_add(
                lv_sb[:].rearrange("p (r d) -> p r d", r=R),
                cat_ps[:, :, 1, :],
                bl_sb[:].rearrange("p (r d) -> p r d", r=R),
            )
            std_sb = outs.tile([128, R * D_OUT], BF16, tag="std")
            nc.scalar.activation(
                std_sb[:], lv_sb[:], mybir.ActivationFunctionType.Exp, scale=0.5
            )
            se_sb = outs.tile([128, R * D_OUT], BF16, tag="se")
            nc.vector.tensor_mul(se_sb[:], std_sb[:], eps_sb[:])
            z_sb = outs.tile([128, R * D_OUT], BF16, tag="z")
            nc.gpsimd.tensor_add(z_sb[:], mean_sb[:], se_sb[:])

            nc.sync.dma_start(lvv[t], lv_sb[:])
            nc.scalar.dma_start(mv[t], mean_sb[:])
            nc.sync.dma_start(zv[t], z_sb[:])

        pending = None  # (t, cat_ps, eps_sb) of the previous tile

        for t in range(n_tiles):
            x_sb = xin.tile([128, R, D_IN], BF16, tag="x")
            nc.sync.dma_start(x_sb[:], xv[t])
            eps_sb = epool.tile([128, R * D_OUT], BF16, tag="eps")
            nc.scalar.dma_start(eps_sb[:], ev[t])

            # transpose x -> xT [d_in, rows] via DMA XBAR (SBUF->SBUF);
            # free pos r*128+q <-> row q*R+r
            xT_sb = xTp.tile([128, ROWS_PER_TILE], L1_DT, tag="xTs")
            for r in range(R):
                eng = nc.sync if r % 2 == 0 else nc.scalar
                eng.dma_start_transpose(
                    xT_sb[:, r * 128 : (r + 1) * 128], x_sb[:, r, :]
                )

            # layer 1: hT[c] = W_emb[:, c].T @ xT  (c: two 128-wide d_h chunks)
            hT_ps0 = psB.tile([128, ROWS_PER_TILE], F32, tag="hT0")
            hT_ps1 = psB.tile([128, ROWS_PER_TILE], F32, tag="hT1")
            nc.tensor.matmul(
                hT_ps0[:], We_sb[:, 0:128], xT_sb[:], start=True, stop=True
            )
            nc.tensor.matmul(
                hT_ps1[:], We_sb[:, 128:256], xT_sb[:], start=True, stop=True
            )
            hT_sb0 = hTp.tile([128, ROWS_PER_TILE], L2_DT, tag="h0")
            hT_sb1 = hTp.tile([128, ROWS_PER_TILE], L2_DT, tag="h1")
            nc.scalar.activation(
                hT_sb0[:],
                hT_ps0[:],
                mybir.ActivationFunctionType.Relu,
                bias=be_sb[:, 0:1],
            )
            nc.scalar.activation(
                hT_sb1[:],
                hT_ps1[:],
                mybir.ActivationFunctionType.Relu,
                bias=be_sb[:, 1:2],
            )

            # layer 2: per 128-row chunk r, one matmul pair against W_cat
            # cat_ps[:, r, 0, :] = mean rows, cat_ps[:, r, 1, :] = logvar rows
            cat_ps = psC.tile([128, R, 2, D_OUT], F32, tag="cat")
            for r in range(R):
                sl = slice(r * 128, (r + 1) * 128)
                nc.tensor.matmul(
                    cat_ps[:, r], hT_sb0[:, sl], Wcat_sb[:, 0, :],
                    start=True, stop=False,
                )
                nc.tensor.matmul(
                    cat_ps[:, r], hT_sb1[:, sl], Wcat_sb[:, 1, :],
                    start=False, stop=True,
                )

            # epilogue for the PREVIOUS tile (software pipelining: producers
            # for PE are emitted ahead of PE-output consumers on each queue)
            if pending is not None:
                epilogue(*pending)
            pending = (t, cat_ps, eps_sb)

        epilogue(*pending)

    nc.finalize()
    return nc


_NC_CACHE = None


def _get_nc():
    global _NC_CACHE
    if _NC_CACHE is None:
        _NC_CACHE = build_bass()
    return _NC_CACHE


def _run(inputs, trace=False, **kw):
    nc = _get_nc()
    xs = np.ascontiguousarray(
        np.asarray(inputs["x"], dtype=np.float32).astype(ml_dtypes.bfloat16)
    )
    es = np.ascontiguousarray(
        np.asarray(inputs["eps"], dtype=np.float32).astype(ml_dtypes.bfloat16)
    )
    weights = {
        k: np.ascontiguousarray(np.asarray(inputs[k], dtype=np.float32))
        for k in ("W_emb", "b_emb", "W_mean", "W_logvar")
    }
    bm = np.asarray(inputs["b_mean"], dtype=np.float32)
    bl = np.asarray(inputs["b_logvar"], dtype=np.float32)
    weights["bm_rep"] = np.ascontiguousarray(np.tile(bm, (128, R)))
    weights["bl_rep"] = np.ascontiguousarray(np.tile(bl, (128, R)))
    in_maps = []
    for c in range(N_CORES):
        sl = slice(c * ROWS_PER_CORE, (c + 1) * ROWS_PER_CORE)
        in_maps.append({"x": xs[sl], "eps": es[sl], **weights})
    res = run_bass_kernel_spmd(nc, in_maps, list(range(N_CORES)), trace=trace, **kw)

    def cat(name):
        return np.concatenate(
            [np.asarray(res.results[c][name]) for c in range(N_CORES)], axis=0
        ).astype(np.float32)

    return (cat("z"), cat("mean"), cat("logvar")), res


def kernel(**inputs):
    out, _ = _run(inputs, trace=False)
    return out


if __name__ == "__main__":
    rng = np.random.default_rng(0)
    demo = {
        "x": rng.standard_normal((B, D_IN), dtype=np.float32),
        "eps": rng.standard_normal((B, D_OUT), dtype=np.float32),
        "W_emb": rng.standard_normal((D_IN, D_H), dtype=np.float32) * 0.088,
        "b_emb": rng.standard_normal((D_H,), dtype=np.float32) * 0.05,
        "W_mean": rng.standard_normal((D_H, D_OUT), dtype=np.float32) * 0.06,
        "b_mean": rng.standard_normal((D_OUT,), dtype=np.float32) * 0.03,
        "W_logvar": rng.standard_normal((D_H, D_OUT), dtype=np.float32) * 0.06,
        "b_logvar": rng.standard_normal((D_OUT,), dtype=np.float32) * 0.03,
    }
    z, m, l = kernel(**demo)
    print("shapes", z.shape, m.shape, l.shape)


# revision 20
# speedup vs baseline: 3.2092x; 3.2092x over previous
"""GaussianMLP sampling kernel for 8 trn2 NeuronCores (pure data parallel).

reference:
    h      = relu(x @ W_emb + b_emb)        x:[B,128] W_emb:[128,256]
    mean   = h @ W_mean + b_mean            W_mean:[256,128]
    logvar = h @ W_logvar + b_logvar        W_logvar:[256,128]
    z      = mean + exp(0.5*logvar) * eps
    returns (z, mean, logvar)

Sharding: x/eps split along batch across 8 cores; weights replicated.

Per-core dataflow (ROWS_PER_TILE=512 rows/iteration, row = t*512 + p*4 + r so
every DMA moves 2KiB-contiguous chunks per partition):
  - DMA x tile [128p, 4, 128] (partition p holds 4 consecutive rows)
  - PE transpose 4x [128,128] -> xT [d_in=128p, 512 rows] in PSUM, DVE copy
    to SBUF as bf16
  - hT = W_emb.T @ x.T via 2 matmuls (lhsT=W_emb chunk, rhs=xT) -> PSUM
  - ACT relu(hT + b_emb) PSUM->SBUF bf16 (per-partition bias)
  - L2: one [128, 2*128] matmul pair per 128-row chunk against
    W_cat = [W_mean | W_logvar] -> cat PSUM [128, (r, head, d)]
  - epilogue: DVE mean/logvar = cat + bias (bias tiles precomputed on host,
    replicated across partitions); ACT std = exp(0.5*logvar); Pool se=std*eps,
    z=mean+se; DMA 3 outputs (2KiB contiguous per partition)
"""

import sys

sys.path.insert(0, "/opt/trn_rl_repo")

import ml_dtypes
import numpy as np

from contextlib import ExitStack

from concourse import bacc, bass, masks, mybir, tile
from concourse.bass_utils import run_bass_kernel_spmd

N_CORES = 8
B = 524288
D_IN = 128
D_H = 256
D_OUT = 128
ROWS_PER_CORE = B // N_CORES  # 65536
ROWS_PER_TILE = 512
N_TILES = ROWS_PER_CORE // ROWS_PER_TILE  # 128
R = ROWS_PER_TILE // 128  # 4 rows per partition per tile

F32 = mybir.dt.float32
BF16 = mybir.dt.bfloat16
L1_DT = BF16
L2_DT = BF16


def build_bass(rows_per_core=ROWS_PER_CORE):
    nc = bacc.Bacc("TRN2", target_bir_lowering=False, debug=False)
    n_tiles = rows_per_core // ROWS_PER_TILE

    x_ext = nc.declare_dram_parameter("x", [rows_per_core, D_IN], BF16, isOutput=False)
    eps_ext = nc.declare_dram_parameter(
        "eps", [rows_per_core, D_OUT], BF16, isOutput=False
    )
    We_ext = nc.declare_dram_parameter("W_emb", [D_IN, D_H], F32, isOutput=False)
    be_ext = nc.declare_dram_parameter("b_emb", [D_H], F32, isOutput=False)
    Wm_ext = nc.declare_dram_parameter("W_mean", [D_H, D_OUT], F32, isOutput=False)
    Wl_ext = nc.declare_dram_parameter("W_logvar", [D_H, D_OUT], F32, isOutput=False)
    # bias rows replicated across 128 partitions and R row-slots (host-prepped)
    bm_rep_ext = nc.declare_dram_parameter(
        "bm_rep", [128, R * D_OUT], F32, isOutput=False
    )
    bl_rep_ext = nc.declare_dram_parameter(
        "bl_rep", [128, R * D_OUT], F32, isOutput=False
    )
    z_ext = nc.declare_dram_parameter("z", [rows_per_core, D_OUT], BF16, isOutput=True)
    mean_ext = nc.declare_dram_parameter(
        "mean", [rows_per_core, D_OUT], BF16, isOutput=True
    )
    lv_ext = nc.declare_dram_parameter(
        "logvar", [rows_per_core, D_OUT], BF16, isOutput=True
    )

    # tiled DRAM views: row = t*ROWS_PER_TILE + p*R + r  (2KiB/partition DMAs)
    xv = x_ext.rearrange("(t p r) d -> t p r d", p=128, r=R)
    ev = eps_ext.rearrange("(t p r) d -> t p (r d)", p=128, r=R)
    zv = z_ext.rearrange("(t p r) d -> t p (r d)", p=128, r=R)
    mv = mean_ext.rearrange("(t p r) d -> t p (r d)", p=128, r=R)
    lvv = lv_ext.rearrange("(t p r) d -> t p (r d)", p=128, r=R)

    with tile.TileContext(nc) as tc, ExitStack() as ctx:
        const = ctx.enter_context(tc.tile_pool(name="const", bufs=1))
        xin = ctx.enter_context(tc.tile_pool(name="xin", bufs=4))
        epool = ctx.enter_context(tc.tile_pool(name="eps", bufs=4))
        xTp = ctx.enter_context(tc.tile_pool(name="xT", bufs=3))
        hTp = ctx.enter_context(tc.tile_pool(name="hTs", bufs=3))
        outs = ctx.enter_context(tc.tile_pool(name="outs", bufs=4))
        psA = ctx.enter_context(tc.tile_pool(name="psA", bufs=2, space="PSUM"))
        psB = ctx.enter_context(tc.tile_pool(name="psB", bufs=1, space="PSUM"))
        psC = ctx.enter_context(tc.tile_pool(name="psC", bufs=2, space="PSUM"))

        # --- constants / weights (loaded once) ---
        ident = const.tile([128, 128], BF16)
        masks.make_identity(nc, ident[:])

        We_sb = const.tile([128, D_H], L1_DT)
        nc.gpsimd.dma_start(We_sb[:], We_ext[:])

        # W_cat[:, c, 0:128] = W_mean chunk c, [:, c, 128:256] = W_logvar chunk c
        Wcat_sb = const.tile([128, 2, 2 * D_OUT], L2_DT)
        nc.gpsimd.dma_start(
            Wcat_sb[:, :, 0:D_OUT], Wm_ext.rearrange("(c p) d -> p c d", p=128)
        )
        nc.gpsimd.dma_start(
            Wcat_sb[:, :, D_OUT : 2 * D_OUT],
            Wl_ext.rearrange("(c p) d -> p c d", p=128),
        )

        be_sb = const.tile([128, 2], F32)
        nc.sync.dma_start(be_sb[:], be_ext.rearrange("(c p) -> p c", p=128))

        bm_sb = const.tile([128, R * D_OUT], F32)
        nc.sync.dma_start(bm_sb[:], bm_rep_ext[:])
        bl_sb = const.tile([128, R * D_OUT], F32)
        nc.sync.dma_start(bl_sb[:], bl_rep_ext[:])

        def epilogue(t, cat_ps, eps_sb):
            mean_sb = outs.tile([128, R * D_OUT], BF16, tag="means")
            nc.vector.tensor_add(
                mean_sb[:].rearrange("p (r d) -> p r d", r=R),
                cat_ps[:, :, 0, :],
                bm_sb[:].rearrange("p (r d) -> p r d", r=R),
            )
            lv_sb = outs.tile([128, R * D_OUT], BF16, tag="lvs")
            nc.vector.tensor_add(
                lv_sb[:].rearrange("p (r d) -> p r d", r=R),
                cat_ps[:, :, 1, :],
                bl_sb[:].rearrange("p (r d) -> p r d", r=R),
            )
            std_sb = outs.tile([128, R * D_OUT], BF16, tag="std")
            nc.scalar.activation(
                std_sb[:], lv_sb[:], mybir.ActivationFunctionType.Exp, scale=0.5
            )
            se_sb = outs.tile([128, R * D_OUT], BF16, tag="se")
            nc.vector.tensor_mul(se_sb[:], std_sb[:], eps_sb[:])
            z_sb = outs.tile([128, R * D_OUT], BF16, tag="z")
            nc.gpsimd.tensor_add(z_sb[:], mean_sb[:], se_sb[:])

            nc.sync.dma_start(lvv[t], lv_sb[:])
            nc.scalar.dma_start(mv[t], mean_sb[:])
            nc.sync.dma_start(zv[t], z_sb[:])

        pending = None  # (t, cat_ps, eps_sb) of the previous tile

        for t in range(n_tiles):
            x_sb = xin.tile([128, R, D_IN], BF16, tag="x")
            nc.sync.dma_start(x_sb[:], xv[t])
            eps_sb = epool.tile([128, R * D_OUT], BF16, tag="eps")
            nc.scalar.dma_start(eps_sb[:], ev[t])

            # transpose x -> xT [d_in, rows]; free pos r*128+q <-> row q*R+r
            xT_ps = psA.tile([128, ROWS_PER_TILE], BF16, tag="xT")
            for r in range(R):
                nc.tensor.transpose(
                    xT_ps[:, r * 128 : (r + 1) * 128], x_sb[:, r, :], ident[:]
                )
            xT_sb = xTp.tile([128, ROWS_PER_TILE], L1_DT, tag="xTs")
            nc.vector.tensor_copy(xT_sb[:], xT_ps[:])

            # layer 1: hT[c] = W_emb[:, c].T @ xT  (c: two 128-wide d_h chunks)
            hT_ps0 = psB.tile([128, ROWS_PER_TILE], F32, tag="hT0")
            hT_ps1 = psB.tile([128, ROWS_PER_TILE], F32, tag="hT1")
            nc.tensor.matmul(
                hT_ps0[:], We_sb[:, 0:128], xT_sb[:], start=True, stop=True
            )
            nc.tensor.matmul(
                hT_ps1[:], We_sb[:, 128:256], xT_sb[:], start=True, stop=True
            )
            hT_sb0 = hTp.tile([128, ROWS_PER_TILE], L2_DT, tag="h0")
            hT_sb1 = hTp.tile([128, ROWS_PER_TILE], L2_DT, tag="h1")
            nc.scalar.activation(
                hT_sb0[:],
                hT_ps0[:],
                mybir.ActivationFunctionType.Relu,
                bias=be_sb[:, 0:1],
            )
            nc.scalar.activation(
                hT_sb1[:],
                hT_ps1[:],
                mybir.ActivationFunctionType.Relu,
                bias=be_sb[:, 1:2],
            )

            # layer 2: per 128-row chunk r, one matmul pair against W_cat
            # cat_ps[:, r, 0, :] = mean rows, cat_ps[:, r, 1, :] = logvar rows
            cat_ps = psC.tile([128, R, 2, D_OUT], F32, tag="cat")
            for r in range(R):
                sl = slice(r * 128, (r + 1) * 128)
                nc.tensor.matmul(
                    cat_ps[:, r], hT_sb0[:, sl], Wcat_sb[:, 0, :],
                    start=True, stop=False,
                )
                nc.tensor.matmul(
                    cat_ps[:, r], hT_sb1[:, sl], Wcat_sb[:, 1, :],
                    start=False, stop=True,
                )

            # epilogue for the PREVIOUS tile (software pipelining: producers
            # for PE are emitted ahead of PE-output consumers on each queue)
            if pending is not None:
                epilogue(*pending)
            pending = (t, cat_ps, eps_sb)

        epilogue(*pending)

    nc.finalize()
    return nc


_NC_CACHE = None


def _get_nc():
    global _NC_CACHE
    if _NC_CACHE is None:
        _NC_CACHE = build_bass()
    return _NC_CACHE


def _run(inputs, trace=False, **kw):
    nc = _get_nc()
    xs = np.ascontiguousarray(
        np.asarray(inputs["x"], dtype=np.float32).astype(ml_dtypes.bfloat16)
    )
    es = np.ascontiguousarray(
        np.asarray(inputs["eps"], dtype=np.float32).astype(ml_dtypes.bfloat16)
    )
    weights = {
        k: np.ascontiguousarray(np.asarray(inputs[k], dtype=np.float32))
        for k in ("W_emb", "b_emb", "W_mean", "W_logvar")
    }
    bm = np.asarray(inputs["b_mean"], dtype=np.float32)
    bl = np.asarray(inputs["b_logvar"], dtype=np.float32)
    weights["bm_rep"] = np.ascontiguousarray(np.tile(bm, (128, R)))
    weights["bl_rep"] = np.ascontiguousarray(np.tile(bl, (128, R)))
    in_maps = []
    for c in range(N_CORES):
        sl = slice(c * ROWS_PER_CORE, (c + 1) * ROWS_PER_CORE)
        in_maps.append({"x": xs[sl], "eps": es[sl], **weights})
    res = run_bass_kernel_spmd(nc, in_maps, list(range(N_CORES)), trace=trace, **kw)

    def cat(name):
        return np.concatenate(
            [np.asarray(res.results[c][name]) for c in range(N_CORES)], axis=0
        ).astype(np.float32)

    return (cat("z"), cat("mean"), cat("logvar")), res


def kernel(**inputs):
    out, _ = _run(inputs, trace=False)
    return out


if __name__ == "__main__":
    rng = np.random.default_rng(0)
    demo = {
        "x": rng.standard_normal((B, D_IN), dtype=np.float32),
        "eps": rng.standard_normal((B, D_OUT), dtype=np.float32),
        "W_emb": rng.standard_normal((D_IN, D_H), dtype=np.float32) * 0.088,
        "b_emb": rng.standard_normal((D_H,), dtype=np.float32) * 0.05,
        "W_mean": rng.standard_normal((D_H, D_OUT), dtype=np.float32) * 0.06,
        "b_mean": rng.standard_normal((D_OUT,), dtype=np.float32) * 0.03,
        "W_logvar": rng.standard_normal((D_H, D_OUT), dtype=np.float32) * 0.06,
        "b_logvar": rng.standard_normal((D_OUT,), dtype=np.float32) * 0.03,
    }
    z, m, l = kernel(**demo)
    print("shapes", z.shape, m.shape, l.shape)


# revision 25
# speedup vs baseline: 3.6959x; 1.1517x over previous
"""GaussianMLP sampling kernel for 8 trn2 NeuronCores (pure data parallel).

reference:
    h      = relu(x @ W_emb + b_emb)        x:[B,128] W_emb:[128,256]
    mean   = h @ W_mean + b_mean            W_mean:[256,128]
    logvar = h @ W_logvar + b_logvar        W_logvar:[256,128]
    z      = mean + exp(0.5*logvar) * eps
    returns (z, mean, logvar)

Sharding: x/eps split along batch across 8 cores; weights replicated.

Per-core dataflow (ROWS_PER_TILE=512 rows/iteration, row = t*512 + p*4 + r so
every DMA moves 2KiB-contiguous chunks per partition):
  - DMA x tile [128p, 4, 128] (partition p holds 4 consecutive rows)
  - PE transpose 4x [128,128] -> xT [d_in=128p, 512 rows] in PSUM, DVE copy
    to SBUF as bf16
  - hT = W_emb.T @ x.T via 2 matmuls (lhsT=W_emb chunk, rhs=xT) -> PSUM
  - ACT relu(hT + b_emb) PSUM->SBUF bf16 (per-partition bias)
  - L2: one [128, 2*128] matmul pair per 128-row chunk against
    W_cat = [W_mean | W_logvar] -> cat PSUM [128, (r, head, d)]
  - epilogue: DVE mean/logvar = cat + bias (bias tiles precomputed on host,
    replicated across partitions); ACT std = exp(0.5*logvar); Pool se=std*eps,
    z=mean+se; DMA 3 outputs (2KiB contiguous per partition)
"""

import sys

sys.path.insert(0, "/opt/trn_rl_repo")

import ml_dtypes
import numpy as np

from contextlib import ExitStack

from concourse import bacc, bass, masks, mybir, tile
from concourse.bass_utils import run_bass_kernel_spmd

N_CORES = 8
B = 524288
D_IN = 128
D_H = 256
D_OUT = 128
ROWS_PER_CORE = B // N_CORES  # 65536
ROWS_PER_TILE = 1024  # supertile: 8 rows per partition, 2KiB bf16 DMA chunks
N_TILES = ROWS_PER_CORE // ROWS_PER_TILE  # 64
RS = ROWS_PER_TILE // 128  # 8 rows per partition per supertile
HALF = 512  # rows per compute half (PSUM-sized)
R = 4  # rows per partition per half (bias replication factor)

F32 = mybir.dt.float32
BF16 = mybir.dt.bfloat16
L1_DT = BF16
L2_DT = BF16


def build_bass(rows_per_core=ROWS_PER_CORE):
    nc = bacc.Bacc("TRN2", target_bir_lowering=False, debug=False)
    n_tiles = rows_per_core // ROWS_PER_TILE

    x_ext = nc.declare_dram_parameter("x", [rows_per_core, D_IN], BF16, isOutput=False)
    eps_ext = nc.declare_dram_parameter(
        "eps", [rows_per_core, D_OUT], BF16, isOutput=False
    )
    We_ext = nc.declare_dram_parameter("W_emb", [D_IN, D_H], F32, isOutput=False)
    be_ext = nc.declare_dram_parameter("b_emb", [D_H], F32, isOutput=False)
    Wm_ext = nc.declare_dram_parameter("W_mean", [D_H, D_OUT], F32, isOutput=False)
    Wl_ext = nc.declare_dram_parameter("W_logvar", [D_H, D_OUT], F32, isOutput=False)
    # bias rows replicated across 128 partitions and R row-slots (host-prepped)
    bm_rep_ext = nc.declare_dram_parameter(
        "bm_rep", [128, R * D_OUT], F32, isOutput=False
    )
    bl_rep_ext = nc.declare_dram_parameter(
        "bl_rep", [128, R * D_OUT], F32, isOutput=False
    )
    z_ext = nc.declare_dram_parameter("z", [rows_per_core, D_OUT], BF16, isOutput=True)
    mean_ext = nc.declare_dram_parameter(
        "mean", [rows_per_core, D_OUT], BF16, isOutput=True
    )
    lv_ext = nc.declare_dram_parameter(
        "logvar", [rows_per_core, D_OUT], BF16, isOutput=True
    )

    # tiled DRAM views: row = t*ROWS_PER_TILE + p*RS + r  (2KiB/partition DMAs)
    xv = x_ext.rearrange("(t p r) d -> t p r d", p=128, r=RS)
    ev = eps_ext.rearrange("(t p r) d -> t p (r d)", p=128, r=RS)
    zv = z_ext.rearrange("(t p r) d -> t p (r d)", p=128, r=RS)
    mv = mean_ext.rearrange("(t p r) d -> t p (r d)", p=128, r=RS)
    lvv = lv_ext.rearrange("(t p r) d -> t p (r d)", p=128, r=RS)

    with tile.TileContext(nc) as tc, ExitStack() as ctx:
        const = ctx.enter_context(tc.tile_pool(name="const", bufs=1))
        xin = ctx.enter_context(tc.tile_pool(name="xin", bufs=4))
        epool = ctx.enter_context(tc.tile_pool(name="eps", bufs=4))
        xTp = ctx.enter_context(tc.tile_pool(name="xT", bufs=3))
        hTp = ctx.enter_context(tc.tile_pool(name="hTs", bufs=3))
        outs = ctx.enter_context(tc.tile_pool(name="outs", bufs=4))
        psA = ctx.enter_context(tc.tile_pool(name="psA", bufs=2, space="PSUM"))
        psB = ctx.enter_context(tc.tile_pool(name="psB", bufs=1, space="PSUM"))
        psC = ctx.enter_context(tc.tile_pool(name="psC", bufs=1, space="PSUM"))

        # --- constants / weights (loaded once) ---
        ident = const.tile([128, 128], BF16)
        masks.make_identity(nc, ident[:])

        We_sb = const.tile([128, D_H], L1_DT)
        nc.gpsimd.dma_start(We_sb[:], We_ext[:])

        # W_cat[:, c, 0:128] = W_mean chunk c, [:, c, 128:256] = W_logvar chunk c
        Wcat_sb = const.tile([128, 2, 2 * D_OUT], L2_DT)
        nc.gpsimd.dma_start(
            Wcat_sb[:, :, 0:D_OUT], Wm_ext.rearrange("(c p) d -> p c d", p=128)
        )
        nc.gpsimd.dma_start(
            Wcat_sb[:, :, D_OUT : 2 * D_OUT],
            Wl_ext.rearrange("(c p) d -> p c d", p=128),
        )

        be_sb = const.tile([128, 2], F32)
        nc.sync.dma_start(be_sb[:], be_ext.rearrange("(c p) -> p c", p=128))

        bm_sb = const.tile([128, R * D_OUT], F32)
        nc.sync.dma_start(bm_sb[:], bm_rep_ext[:])
        bl_sb = const.tile([128, R * D_OUT], F32)
        nc.sync.dma_start(bl_sb[:], bl_rep_ext[:])

        bm_v = bm_sb[:].rearrange("p (r d) -> p r d", r=R)
        bl_v = bl_sb[:].rearrange("p (r d) -> p r d", r=R)

        for t in range(n_tiles):
            x_sb = xin.tile([128, RS, D_IN], BF16, tag="x")
            nc.sync.dma_start(x_sb[:], xv[t])
            eps_sb = epool.tile([128, RS * D_OUT], BF16, tag="eps")
            nc.scalar.dma_start(eps_sb[:], ev[t])

            # transpose x -> xT [d_in, rows]; free pos r*128+q <-> row q*RS+r
            xT_ps = psA.tile([128, ROWS_PER_TILE], BF16, tag="xT")
            for r in range(RS):
                nc.tensor.transpose(
                    xT_ps[:, r * 128 : (r + 1) * 128], x_sb[:, r, :], ident[:]
                )
            xT_sb = xTp.tile([128, ROWS_PER_TILE], L1_DT, tag="xTs")
            nc.vector.tensor_copy(xT_sb[:], xT_ps[:])

            # layer 1: hT[c] = W_emb[:, c].T @ xT, in two N=512 pieces per chunk
            hT_ps0 = psB.tile([128, ROWS_PER_TILE], F32, tag="hT0")
            hT_ps1 = psB.tile([128, ROWS_PER_TILE], F32, tag="hT1")
            for h in range(2):
                s = slice(h * HALF, (h + 1) * HALF)
                nc.tensor.matmul(
                    hT_ps0[:, s], We_sb[:, 0:128], xT_sb[:, s],
                    start=True, stop=True, skip_group_check=True,
                )
                nc.tensor.matmul(
                    hT_ps1[:, s], We_sb[:, 128:256], xT_sb[:, s],
                    start=True, stop=True, skip_group_check=True,
                )
            hT_sb0 = hTp.tile([128, ROWS_PER_TILE], L2_DT, tag="h0")
            hT_sb1 = hTp.tile([128, ROWS_PER_TILE], L2_DT, tag="h1")
            nc.scalar.activation(
                hT_sb0[:],
                hT_ps0[:],
                mybir.ActivationFunctionType.Relu,
                bias=be_sb[:, 0:1],
            )
            nc.scalar.activation(
                hT_sb1[:],
                hT_ps1[:],
                mybir.ActivationFunctionType.Relu,
                bias=be_sb[:, 1:2],
            )

            # layer 2 + epilogue, one 512-row half at a time
            mean_sb = outs.tile([128, 2, HALF], BF16, tag="means")
            lv_sb = outs.tile([128, 2, HALF], BF16, tag="lvs")
            z_sb = outs.tile([128, 2, HALF], BF16, tag="z")
            for h in range(2):
                # cat_ps[:, j, 0, :] = mean rows, cat_ps[:, j, 1, :] = logvar
                cat_ps = psC.tile([128, R, 2, D_OUT], F32, tag="cat")
                for j in range(R):
                    sl = slice((h * R + j) * 128, (h * R + j + 1) * 128)
                    nc.tensor.matmul(
                        cat_ps[:, j], hT_sb0[:, sl], Wcat_sb[:, 0, :],
                        start=True, stop=False,
                    )
                    nc.tensor.matmul(
                        cat_ps[:, j], hT_sb1[:, sl], Wcat_sb[:, 1, :],
                        start=False, stop=True,
                    )
                nc.vector.tensor_add(
                    mean_sb[:, h].rearrange("p (r d) -> p r d", r=R),
                    cat_ps[:, :, 0, :],
                    bm_v,
                )
                nc.vector.tensor_add(
                    lv_sb[:, h].rearrange("p (r d) -> p r d", r=R),
                    cat_ps[:, :, 1, :],
                    bl_v,
                )
                std_sb = outs.tile([128, HALF], BF16, tag="std")
                nc.scalar.activation(
                    std_sb[:], lv_sb[:, h],
                    mybir.ActivationFunctionType.Exp, scale=0.5,
                )
                se_sb = outs.tile([128, HALF], BF16, tag="se")
                mul_eng = nc.vector if h == 0 else nc.gpsimd
                mul_eng.tensor_mul(
                    se_sb[:], std_sb[:], eps_sb[:, h * HALF : (h + 1) * HALF]
                )
                nc.gpsimd.tensor_add(z_sb[:, h], mean_sb[:, h], se_sb[:])

            nc.sync.dma_start(lvv[t], lv_sb[:].rearrange("p h f -> p (h f)"))
            nc.scalar.dma_start(mv[t], mean_sb[:].rearrange("p h f -> p (h f)"))
            nc.sync.dma_start(zv[t], z_sb[:].rearrange("p h f -> p (h f)"))

    nc.finalize()
    return nc


_NC_CACHE = None


def _get_nc():
    global _NC_CACHE
    if _NC_CACHE is None:
        _NC_CACHE = build_bass()
    return _NC_CACHE


def _run(inputs, trace=False, **kw):
    nc = _get_nc()
    xs = np.ascontiguousarray(
        np.asarray(inputs["x"], dtype=np.float32).astype(ml_dtypes.bfloat16)
    )
    es = np.ascontiguousarray(
        np.asarray(inputs["eps"], dtype=np.float32).astype(ml_dtypes.bfloat16)
    )
    weights = {
        k: np.ascontiguousarray(np.asarray(inputs[k], dtype=np.float32))
        for k in ("W_emb", "b_emb", "W_mean", "W_logvar")
    }
    bm = np.asarray(inputs["b_mean"], dtype=np.float32)
    bl = np.asarray(inputs["b_logvar"], dtype=np.float32)
    weights["bm_rep"] = np.ascontiguousarray(np.tile(bm, (128, R)))
    weights["bl_rep"] = np.ascontiguousarray(np.tile(bl, (128, R)))
    in_maps = []
    for c in range(N_CORES):
        sl = slice(c * ROWS_PER_CORE, (c + 1) * ROWS_PER_CORE)
        in_maps.append({"x": xs[sl], "eps": es[sl], **weights})
    res = run_bass_kernel_spmd(nc, in_maps, list(range(N_CORES)), trace=trace, **kw)

    def cat(name):
        return np.concatenate(
            [np.asarray(res.results[c][name]) for c in range(N_CORES)], axis=0
        ).astype(np.float32)

    return (cat("z"), cat("mean"), cat("logvar")), res


def kernel(**inputs):
    out, _ = _run(inputs, trace=False)
    return out


if __name__ == "__main__":
    rng = np.random.default_rng(0)
    demo = {
        "x": rng.standard_normal((B, D_IN), dtype=np.float32),
        "eps": rng.standard_normal((B, D_OUT), dtype=np.float32),
        "W_emb": rng.standard_normal((D_IN, D_H), dtype=np.float32) * 0.088,
        "b_emb": rng.standard_normal((D_H,), dtype=np.float32) * 0.05,
        "W_mean": rng.standard_normal((D_H, D_OUT), dtype=np.float32) * 0.06,
        "b_mean": rng.standard_normal((D_OUT,), dtype=np.float32) * 0.03,
        "W_logvar": rng.standard_normal((D_H, D_OUT), dtype=np.float32) * 0.06,
        "b_logvar": rng.standard_normal((D_OUT,), dtype=np.float32) * 0.03,
    }
    z, m, l = kernel(**demo)
    print("shapes", z.shape, m.shape, l.shape)
